# revision 12
# baseline (speedup 1.0000x reference)
"""Trainium2 Bass kernel for BarlowTwinsLoss (nn_BarlowTwinsLoss_11038065951192).

Full inputs: e_q, tau [16384, 2048] f32. Output: scalar f32 loss.

Strategy (data-parallel over the batch axis, 8 NeuronCores):
  - each core holds a [2048, 2048] row-shard of e_q and tau
  - one pass over the shard computes 5 per-feature partial sums in PSUM via
    ones-vector matmuls: S1e, S1t, S2e, S2t, Set (each length-2048, chunked
    as 4 x 512 across PSUM banks, one chunk per partition group 0/32/64/96)
  - the 5x2048 f32 stats are AllReduced across the 8 cores (40 KB)
  - a tiny epilogue computes mean/std/diag-corr and the final scalar loss
    identically on every core; core 0's scalar is returned.

The module is self-contained: it builds + compiles the Bass graph on first
call and caches the jitted PJRT executable for repeat calls.
"""

import numpy as np

N_FULL = 16384
D = 2048
N_CORES = 8
N_SHARD = N_FULL // N_CORES  # 2048 rows per core
P = 128
N_TILES = N_SHARD // P  # 16
CHUNK = 512
N_CHUNKS = D // CHUNK  # 4
NSTATS = 5  # S1e, S1t, S2e, S2t, Set
EPS = 1e-9

_CACHE = {}


def _build_nc(repeat=1):
    import concourse.bacc as bacc
    import concourse.tile as tile
    from concourse import mybir

    f32 = mybir.dt.float32
    bf16 = mybir.dt.bfloat16
    Act = mybir.ActivationFunctionType
    Alu = mybir.AluOpType

    nc = bacc.Bacc(
        "TRN2",
        target_bir_lowering=False,
        debug=False,
        enable_asserts=False,
        num_devices=N_CORES,
    )
    eq_d = nc.dram_tensor("e_q", [N_SHARD, D], f32, kind="ExternalInput")
    ta_d = nc.dram_tensor("tau", [N_SHARD, D], f32, kind="ExternalInput")
    out_d = nc.dram_tensor("out", [1, 1], f32, kind="ExternalOutput")

    with tile.TileContext(nc) as tc:
        with (
            tc.tile_pool(name="io", bufs=3) as io,
            tc.tile_pool(name="bfp", bufs=2) as bfp,
            tc.tile_pool(name="misc", bufs=1) as misc,
            tc.tile_pool(name="ep", bufs=1) as ep,
            tc.tile_pool(name="psp", bufs=1, space="PSUM") as psp,
            tc.tile_pool(name="dram", bufs=1, space="DRAM") as dram,
        ):
            ones_bf = misc.tile([P, 1], bf16)
            nc.gpsimd.memset(ones_bf[:], 1.0)
            zero_b = misc.tile([P, 1], f32)
            nc.gpsimd.memset(zero_b[:], 0.0)

            for _rep in range(repeat):
                cc_in = dram.tile(
                    [N_CHUNKS, NSTATS, CHUNK], f32, tag=f"cc_in{_rep}", name="cc_in"
                )
                cc_out = dram.tile(
                    [N_CHUNKS, NSTATS, CHUNK],
                    f32,
                    addr_space="Shared",
                    tag=f"cc_out{_rep}",
                    name="cc_out",
                )
                # stats accumulate in PSUM; matmul outputs may only target
                # partitions {0, 32, 64}, so: partition group g = s // 2,
                # bank = (s % 2) * 4 + c  (stat-pair per group, chunk in bank)
                psum_stats = psp.tile([65, 2 * N_CHUNKS * CHUNK], f32, tag="stats")
                # the matmuls only write rows {0,32,64}; zero the rest so the
                # later whole-tile PSUM->SBUF copy reads initialized memory
                nc.vector.memset(psum_stats[:], 0.0)

                for i in range(N_TILES):
                    e_t = io.tile([P, D], f32, tag="e")
                    t_t = io.tile([P, D], f32, tag="t")
                    nc.sync.dma_start(e_t[:], eq_d[i * P : (i + 1) * P, :])
                    nc.sync.dma_start(t_t[:], ta_d[i * P : (i + 1) * P, :])

                    e_bf = bfp.tile([P, D], bf16, tag="e_bf")
                    t_bf = bfp.tile([P, D], bf16, tag="t_bf")
                    e2_bf = bfp.tile([P, D], bf16, tag="e2_bf")
                    t2_bf = bfp.tile([P, D], bf16, tag="t2_bf")
                    et_bf = bfp.tile([P, D], bf16, tag="et_bf")

                    nc.vector.tensor_copy(e_bf[:], e_t[:])
                    nc.vector.tensor_copy(t_bf[:], t_t[:])
                    nc.scalar.activation(e2_bf[:], e_t[:], Act.Square, bias=zero_b[:])
                    nc.scalar.activation(t2_bf[:], t_t[:], Act.Square, bias=zero_b[:])
                    nc.vector.tensor_mul(et_bf[:], e_bf[:], t_bf[:])

                    for s, src in enumerate((e_bf, t_bf, e2_bf, t2_bf, et_bf)):
                        g, sl = divmod(s, 2)
                        for c in range(N_CHUNKS):
                            col = (sl * N_CHUNKS + c) * CHUNK
                            nc.tensor.matmul(
                                psum_stats[32 * g : 32 * g + 1, col : col + CHUNK],
                                ones_bf[:, 0:1],
                                src[:, c * CHUNK : (c + 1) * CHUNK],
                                start=(i == 0),
                                stop=(i == N_TILES - 1),
                            )

                # PSUM -> SBUF staging (DMA cannot read PSUM). One copy: the
                # cost is the per-partition free size; partitions run in
                # parallel, so copying the unused rows too is free.
                sb_stats = ep.tile([65, 2 * N_CHUNKS * CHUNK], f32, tag="sb_stats")
                nc.vector.tensor_copy(sb_stats[:, : N_CHUNKS * CHUNK],
                                      psum_stats[:, : N_CHUNKS * CHUNK])
                nc.scalar.copy(sb_stats[:, N_CHUNKS * CHUNK :],
                               psum_stats[:, N_CHUNKS * CHUNK :])

                # staged stats -> DRAM bounce for the collective: one DMA per
                # stat, scattering its 4 chunks into cc_in's [chunk, stat, :]
                for s in range(NSTATS):
                    g, sl = divmod(s, 2)
                    src = sb_stats[
                        32 * g : 32 * g + 1,
                        sl * N_CHUNKS * CHUNK : (sl + 1) * N_CHUNKS * CHUNK,
                    ].rearrange("p (c k) -> p c k", c=N_CHUNKS)
                    nc.sync.dma_start(cc_in[:, s, :], src)

                nc.gpsimd.collective_compute(
                    "AllReduce",
                    Alu.add,
                    replica_groups=[list(range(N_CORES))],
                    ins=[cc_in.opt()],
                    outs=[cc_out.opt()],
                )

                st = ep.tile([N_CHUNKS, NSTATS, CHUNK], f32, tag="st")
                nc.sync.dma_start(st[:], cc_out[:])

                # ---- epilogue on [4, 512] tiles (global stats) ----
                A = st[:, 0, :]  # S1e
                B = st[:, 1, :]  # S1t
                C = st[:, 2, :]  # S2e
                Dq = st[:, 3, :]  # S2t
                E = st[:, 4, :]  # Set

                sh = [N_CHUNKS, CHUNK]
                zb = zero_b[0:N_CHUNKS, 0:1]
                me = ep.tile(sh, f32, tag="me")
                mt = ep.tile(sh, f32, tag="mt")
                ve = ep.tile(sh, f32, tag="ve")
                vt = ep.tile(sh, f32, tag="vt")
                stde = ep.tile(sh, f32, tag="stde")
                stdt = ep.tile(sh, f32, tag="stdt")
                cov = ep.tile(sh, f32, tag="cov")
                den = ep.tile(sh, f32, tag="den")
                rec = ep.tile(sh, f32, tag="rec")
                cr = ep.tile(sh, f32, tag="cr")
                ccl = ep.tile(sh, f32, tag="ccl")
                rr = ep.tile(sh, f32, tag="rr")
                r2 = ep.tile(sh, f32, tag="r2")
                ls = ep.tile([N_CHUNKS, 1], f32, tag="ls")

                inv_n = 1.0 / N_FULL
                # means
                nc.vector.tensor_scalar_mul(me[:], A, inv_n)
                nc.vector.tensor_scalar_mul(mt[:], B, inv_n)
                # sum((x-mean)^2) = S2 - S1*mean ; std = max(sqrt(./(N-1)), eps)
                nc.vector.tensor_mul(ve[:], A, me[:])
                nc.vector.tensor_sub(ve[:], C, ve[:])
                nc.scalar.activation(
                    stde[:], ve[:], Act.Sqrt, bias=zb, scale=1.0 / (N_FULL - 1)
                )
                nc.vector.tensor_scalar_max(stde[:], stde[:], EPS)
                nc.vector.tensor_mul(vt[:], B, mt[:])
                nc.vector.tensor_sub(vt[:], Dq, vt[:])
                nc.scalar.activation(
                    stdt[:], vt[:], Act.Sqrt, bias=zb, scale=1.0 / (N_FULL - 1)
                )
                nc.vector.tensor_scalar_max(stdt[:], stdt[:], EPS)
                # cov = Set - S1e*mt ; c = cov / (stde*stdt) / (N+eps)
                nc.vector.tensor_mul(cov[:], A, mt[:])
                nc.vector.tensor_sub(cov[:], E, cov[:])
                nc.vector.tensor_mul(den[:], stde[:], stdt[:])
                nc.vector.reciprocal(rec[:], den[:])
                nc.vector.scalar_tensor_tensor(
                    cr[:], cov[:], 1.0 / (N_FULL + EPS), rec[:], Alu.mult, Alu.mult
                )
                # clip, r = 1 - c, partial loss = sum(r^2) per partition
                nc.vector.tensor_scalar(
                    ccl[:], cr[:], -1.0 + EPS, 1.0 - EPS, Alu.max, Alu.min
                )
                nc.vector.tensor_scalar(rr[:], ccl[:], -1.0, 1.0, Alu.mult, Alu.add)
                # (tensor_tensor_reduce crashes the exec unit on this stack;
                # use a plain multiply + free-axis reduction instead)
                nc.vector.tensor_mul(r2[:], rr[:], rr[:])
                nc.vector.reduce_sum(ls[:], r2[:], axis=mybir.AxisListType.X)
                # sum the 4 per-partition partials -> scalar: bounce the [4,1]
                # column through DRAM to land it on one partition, then reduce
                # along the free axis.
                ls_dram = dram.tile([N_CHUNKS], f32, tag="ls_dram")
                nc.sync.dma_start(ls_dram[:], ls[:])
                lsT = ep.tile([1, N_CHUNKS], f32, tag="lsT")
                nc.sync.dma_start(lsT[:], ls_dram[:])
                loss_sb = ep.tile([1, 1], f32, tag="loss_sb")
                nc.vector.reduce_sum(
                    loss_sb[:], lsT[:], axis=mybir.AxisListType.X
                )
                nc.sync.dma_start(out_d[:], loss_sb[:])

    nc.compile()
    return nc


class _Exec:
    """Cached PJRT executable for the SPMD kernel (mirrors
    concourse.bass2jax.run_bass_via_pjrt's multi-core branch, but keeps the
    jitted callable so repeat invocations don't recompile)."""

    def __init__(self, nc):
        import jax
        from jax.experimental.shard_map import shard_map
        from jax.sharding import Mesh, PartitionSpec

        from concourse import bass2jax, mybir

        bass2jax.install_neuronx_cc_hook()
        self.nc = nc
        partition_name = (
            nc.partition_id_tensor.name if nc.partition_id_tensor else None
        )

        in_names, out_names, out_avals, zero_outs = [], [], [], []
        for alloc in nc.m.functions[0].allocations:
            if not isinstance(alloc, mybir.MemoryLocationSet):
                continue
            assert alloc.memorylocations
            name = alloc.memorylocations[0].name
            if alloc.kind == "ExternalInput":
                if name != partition_name:
                    in_names.append(name)
            elif alloc.kind == "ExternalOutput":
                shape = tuple(alloc.tensor_shape)
                dtype = mybir.dt.np(alloc.dtype)
                out_names.append(name)
                out_avals.append(jax.core.ShapedArray(shape, dtype))
                zero_outs.append(np.zeros(shape, dtype))

        self.in_names = list(in_names)
        self.out_names = list(out_names)
        self.out_avals = out_avals
        self.zero_outs = zero_outs
        n_params = len(in_names)
        n_outs = len(out_names)

        in_names_full = list(in_names) + list(out_names)
        if partition_name is not None:
            in_names_full.append(partition_name)

        def _body(*args):
            operands = list(args)
            if partition_name is not None:
                operands.append(bass2jax.partition_id_tensor())
            outs = bass2jax._bass_exec_p.bind(
                *operands,
                out_avals=tuple(out_avals),
                in_names=tuple(in_names_full),
                out_names=tuple(out_names),
                lowering_input_output_aliases=(),
                sim_require_finite=True,
                sim_require_nnan=True,
                nc=nc,
            )
            return tuple(outs)

        devices = jax.devices()[:N_CORES]
        assert len(devices) == N_CORES, f"need {N_CORES} devices, got {len(devices)}"
        self.mesh = Mesh(np.asarray(devices), ("core",))
        in_specs = (PartitionSpec("core"),) * (n_params + n_outs)
        out_specs = (PartitionSpec("core"),) * n_outs
        donate = tuple(range(n_params, n_params + n_outs))
        self.sharded = jax.jit(
            shard_map(
                _body,
                mesh=self.mesh,
                in_specs=in_specs,
                out_specs=out_specs,
                check_rep=False,
            ),
            donate_argnums=donate,
            keep_unused=True,
        )

    def concat_zeros(self):
        return [
            np.zeros((N_CORES * z.shape[0], *z.shape[1:]), z.dtype)
            for z in self.zero_outs
        ]

    def run(self, in_map):
        """in_map: name -> full (already concat-along-axis0) array."""
        ins = [in_map[name] for name in self.in_names]
        outs = self.sharded(*ins, *self.concat_zeros())
        return {
            name: np.asarray(outs[i]).reshape(
                N_CORES, *self.out_avals[i].shape
            )
            for i, name in enumerate(self.out_names)
        }


def _get_exec(repeat=1):
    key = ("exec", repeat)
    if key not in _CACHE:
        _CACHE[key] = _Exec(_build_nc(repeat))
    return _CACHE[key]


def kernel(e_q, tau):
    e_q = np.ascontiguousarray(np.asarray(e_q), dtype=np.float32)
    tau = np.ascontiguousarray(np.asarray(tau), dtype=np.float32)
    assert e_q.shape == (N_FULL, D) and tau.shape == (N_FULL, D)
    ex = _get_exec()
    # row-sharding across cores: the concatenation of the 8 shards along
    # axis 0 is just the full array, so pass it through unchanged.
    outs = ex.run({"e_q": e_q, "tau": tau})
    loss = outs["out"][0, 0, 0]  # identical on every core post-AllReduce
    return np.asarray(loss, dtype=np.float32)
